# revision 20
# baseline (speedup 1.0000x reference)
"""BPMLL loss kernel for Trainium2, data-parallel over 8 NeuronCores.

Reference computation (per sample row i of c [B, L], y [B, L] in {0,1}):
    pos_i  = sum_l y_il * exp(-c_il)
    neg_i  = sum_l (1 - y_il) * exp(c_il)
    Sy_i   = sum_l y_il
    loss_i = pos_i * neg_i / (Sy_i * (L - Sy_i))
    out    = mean_i loss_i                      (scalar, float32)

Device strategy: shard the batch dim across 8 cores (2048 rows each). The
label masking is folded into the exponent: with s = M*y - c and M = 128,
    exp(-s)     = exp(c - M*y)     -> (1-y)*exp(c)   (y=1 underflows to 0)
    exp(s - M)  = exp(-c + M*(y-1))-> y*exp(-c)      (y=0 underflows to 0)
so ScalarE's fused activation-with-accumulate computes each masked row sum
in a single pass.

The host packs each [128, 1024] row-tile pair into one contiguous block:
per partition row, 4096 B of c (f32) followed by 1024 B of y (int8 - the
mask is 0/1 so the downcast is lossless and cuts DMA bytes by 37%). Each
tile arrives in a single 640 KB SWDGE DMA; the kernel bitcasts the two
regions back to f32 / int8 on-chip. Per tile the device does: one DVE
scalar_tensor_tensor (s = y*M - c), one DVE reduce_sum over y, and two
ScalarE exp+accum passes. Each core emits [3, 128, 16] row statistics
(pos, neg, Sy); the host finishes the tiny per-row division and the
global mean in float64.
"""

import numpy as np

B, L = 16384, 1024
N_CORES = 8
BS = B // N_CORES  # 2048 rows per core
P = 128
NSEG = BS // P  # 16 tiles of [128, L] per core
MASK = 128.0
ROWB = 4 * L + L  # bytes per partition row: c (f32) + y (int8)
DGE = "gpsimd"  # which engine issues the input loads: "gpsimd" or "sync"
IO_BUFS = 5


def _build_nc():
    import concourse.bacc as bacc
    import concourse.mybir as mybir
    from concourse.tile import TileContext

    f32 = mybir.dt.float32
    i8 = mybir.dt.int8
    u8 = mybir.dt.uint8

    nc = bacc.Bacc()
    cy_in = nc.dram_tensor("cy", [NSEG, P, ROWB], u8, kind="ExternalInput")
    stats = nc.dram_tensor("stats", [3, P, NSEG], f32, kind="ExternalOutput")

    with TileContext(nc) as tc:
        with (
            tc.tile_pool(name="io", bufs=IO_BUFS) as io,
            tc.tile_pool(name="sbuf_s", bufs=NSEG) as spool,
            tc.tile_pool(name="scratch", bufs=1) as scratch,
            tc.tile_pool(name="accs", bufs=1) as accs,
        ):
            pos = accs.tile([P, NSEG], f32)
            neg = accs.tile([P, NSEG], f32)
            ysum = accs.tile([P, NSEG], f32)
            neg_mask = accs.tile([P, 1], f32)
            nc.vector.memset(neg_mask[:], -MASK)
            # Both exp passes dump their (unused) elementwise output here;
            # same-engine WAW ordering needs no semaphores.
            scr = scratch.tile([P, L], f32)

            dma_eng = nc.gpsimd if DGE == "gpsimd" else nc.sync
            for i in range(NSEG):
                t = io.tile([P, ROWB], u8, tag="cy")
                dma_eng.dma_start(t[:], cy_in[i])
                c_ap = t[:, 0 : 4 * L].bitcast(f32)
                y_ap = t[:, 4 * L : ROWB].bitcast(i8)

                s = spool.tile([P, L], f32, tag="s")
                nc.vector.scalar_tensor_tensor(
                    s[:],
                    y_ap,
                    MASK,
                    c_ap,
                    mybir.AluOpType.mult,
                    mybir.AluOpType.subtract,
                )
                nc.vector.reduce_sum(
                    ysum[:, i : i + 1], y_ap, axis=mybir.AxisListType.X
                )
                nc.scalar.activation(
                    scr[:],
                    s[:],
                    mybir.ActivationFunctionType.Exp,
                    scale=-1.0,
                    accum_out=neg[:, i : i + 1],
                )
                nc.scalar.activation(
                    scr[:],
                    s[:],
                    mybir.ActivationFunctionType.Exp,
                    bias=neg_mask[:],
                    scale=1.0,
                    accum_out=pos[:, i : i + 1],
                )

            nc.sync.dma_start(stats[0], pos[:])
            nc.sync.dma_start(stats[1], neg[:])
            nc.sync.dma_start(stats[2], ysum[:])

    nc.finalize()
    return nc


def _run(nc, in_maps, **kwargs):
    from concourse.bass_utils import run_bass_kernel_spmd

    return run_bass_kernel_spmd(nc, in_maps, list(range(N_CORES)), **kwargs)


def kernel(c, y, _bench_kwargs=None, _bench_result=None):
    c = np.asarray(c, dtype=np.float32)
    y = np.asarray(y, dtype=np.int32)
    assert c.shape == (B, L) and y.shape == (B, L)

    # Pack per [128, L] row-tile: per partition row 4096 B of c then 1024 B
    # of y as int8, so each tile is one contiguous 640 KB DMA.
    cyv = np.empty((N_CORES, NSEG, P, ROWB), np.uint8)
    cb = np.ascontiguousarray(c).view(np.uint8).reshape(N_CORES, NSEG, P, 4 * L)
    cyv[..., : 4 * L] = cb
    cyv[..., 4 * L :] = y.astype(np.uint8).reshape(N_CORES, NSEG, P, L)

    nc = _build_nc()
    in_maps = [{"cy": cyv[k]} for k in range(N_CORES)]
    res = _run(nc, in_maps, **(_bench_kwargs or {}))
    if _bench_result is not None:
        _bench_result.append(res)

    stats = np.stack([r["stats"] for r in res.results])  # [8, 3, 128, 16]
    pos = stats[:, 0].astype(np.float64)
    neg = stats[:, 1].astype(np.float64)
    sy = stats[:, 2].astype(np.float64)
    loss = pos * neg / (sy * (L - sy))
    return np.asarray(loss.mean(), dtype=np.float32)


# revision 21
# speedup vs baseline: 1.0305x; 1.0305x over previous
"""BPMLL loss kernel for Trainium2, data-parallel over 8 NeuronCores.

Reference computation (per sample row i of c [B, L], y [B, L] in {0,1}):
    pos_i  = sum_l y_il * exp(-c_il)
    neg_i  = sum_l (1 - y_il) * exp(c_il)
    Sy_i   = sum_l y_il
    loss_i = pos_i * neg_i / (Sy_i * (L - Sy_i))
    out    = mean_i loss_i                      (scalar, float32)

Device strategy: shard the batch dim across 8 cores (2048 rows each). The
label masking is folded into the exponent: with s = M*y - c and M = 128,
    exp(-s)     = exp(c - M*y)     -> (1-y)*exp(c)   (y=1 underflows to 0)
    exp(s - M)  = exp(-c + M*(y-1))-> y*exp(-c)      (y=0 underflows to 0)
so ScalarE's fused activation-with-accumulate computes each masked row sum
in a single pass.

The host packs each [128, 1024] row-tile pair into one contiguous block:
per partition row, 4096 B of c (f32) followed by 1024 B of y (int8 - the
mask is 0/1 so the downcast is lossless and cuts DMA bytes by 37%). Each
tile arrives in a single 640 KB SWDGE DMA; the kernel bitcasts the two
regions back to f32 / int8 on-chip. Per tile the device does: one DVE
scalar_tensor_tensor (s = y*M - c), one DVE reduce_sum over y, and two
ScalarE exp+accum passes. Each core emits [3, 128, 16] row statistics
(pos, neg, Sy); the host finishes the tiny per-row division and the
global mean in float64.
"""

import numpy as np

B, L = 16384, 1024
N_CORES = 8
BS = B // N_CORES  # 2048 rows per core
P = 128
NSEG = BS // P  # 16 tiles of [128, L] per core
MASK = 128.0
ROWB = 4 * L + L  # bytes per partition row: c (f32) + y (int8)
DGE = "gpsimd"  # which engine issues the input loads: "gpsimd" or "sync"
IO_BUFS = 5


def _build_nc():
    import concourse.bacc as bacc
    import concourse.mybir as mybir
    from concourse.tile import TileContext

    f32 = mybir.dt.float32
    i8 = mybir.dt.int8
    u8 = mybir.dt.uint8

    nc = bacc.Bacc()
    cy_in = nc.dram_tensor("cy", [NSEG, P, ROWB], u8, kind="ExternalInput")
    stats = nc.dram_tensor("stats", [3, P, NSEG], f32, kind="ExternalOutput")

    with TileContext(nc) as tc:
        with (
            tc.tile_pool(name="io", bufs=IO_BUFS) as io,
            tc.tile_pool(name="sbuf_s", bufs=NSEG) as spool,
            tc.tile_pool(name="scratch", bufs=1, space="PSUM") as scratch,
            tc.tile_pool(name="accs", bufs=1) as accs,
        ):
            pos = accs.tile([P, NSEG], f32)
            neg = accs.tile([P, NSEG], f32)
            ysum = accs.tile([P, NSEG], f32)
            neg_mask = accs.tile([P, 1], f32)
            nc.vector.memset(neg_mask[:], -MASK)
            # Both exp passes dump their (unused) elementwise output here;
            # same-engine WAW ordering needs no semaphores.
            scr = scratch.tile([P, L], f32)

            dma_eng = nc.gpsimd if DGE == "gpsimd" else nc.sync
            for i in range(NSEG):
                t = io.tile([P, ROWB], u8, tag="cy")
                dma_eng.dma_start(t[:], cy_in[i])
                c_ap = t[:, 0 : 4 * L].bitcast(f32)
                y_ap = t[:, 4 * L : ROWB].bitcast(i8)

                s = spool.tile([P, L], f32, tag="s")
                nc.vector.scalar_tensor_tensor(
                    s[:],
                    y_ap,
                    MASK,
                    c_ap,
                    mybir.AluOpType.mult,
                    mybir.AluOpType.subtract,
                )
                nc.vector.reduce_sum(
                    ysum[:, i : i + 1], y_ap, axis=mybir.AxisListType.X
                )
                nc.scalar.activation(
                    scr[:],
                    s[:],
                    mybir.ActivationFunctionType.Exp,
                    scale=-1.0,
                    accum_out=neg[:, i : i + 1],
                )
                nc.scalar.activation(
                    scr[:],
                    s[:],
                    mybir.ActivationFunctionType.Exp,
                    bias=neg_mask[:],
                    scale=1.0,
                    accum_out=pos[:, i : i + 1],
                )

            nc.sync.dma_start(stats[0], pos[:])
            nc.sync.dma_start(stats[1], neg[:])
            nc.sync.dma_start(stats[2], ysum[:])

    nc.finalize()
    return nc


def _run(nc, in_maps, **kwargs):
    from concourse.bass_utils import run_bass_kernel_spmd

    return run_bass_kernel_spmd(nc, in_maps, list(range(N_CORES)), **kwargs)


def kernel(c, y, _bench_kwargs=None, _bench_result=None):
    c = np.asarray(c, dtype=np.float32)
    y = np.asarray(y, dtype=np.int32)
    assert c.shape == (B, L) and y.shape == (B, L)

    # Pack per [128, L] row-tile: per partition row 4096 B of c then 1024 B
    # of y as int8, so each tile is one contiguous 640 KB DMA.
    cyv = np.empty((N_CORES, NSEG, P, ROWB), np.uint8)
    cb = np.ascontiguousarray(c).view(np.uint8).reshape(N_CORES, NSEG, P, 4 * L)
    cyv[..., : 4 * L] = cb
    cyv[..., 4 * L :] = y.astype(np.uint8).reshape(N_CORES, NSEG, P, L)

    nc = _build_nc()
    in_maps = [{"cy": cyv[k]} for k in range(N_CORES)]
    res = _run(nc, in_maps, **(_bench_kwargs or {}))
    if _bench_result is not None:
        _bench_result.append(res)

    stats = np.stack([r["stats"] for r in res.results])  # [8, 3, 128, 16]
    pos = stats[:, 0].astype(np.float64)
    neg = stats[:, 1].astype(np.float64)
    sy = stats[:, 2].astype(np.float64)
    loss = pos * neg / (sy * (L - sy))
    return np.asarray(loss.mean(), dtype=np.float32)


# revision 22
# speedup vs baseline: 1.0332x; 1.0026x over previous
"""BPMLL loss kernel for Trainium2, data-parallel over 8 NeuronCores.

Reference computation (per sample row i of c [B, L], y [B, L] in {0,1}):
    pos_i  = sum_l y_il * exp(-c_il)
    neg_i  = sum_l (1 - y_il) * exp(c_il)
    Sy_i   = sum_l y_il
    loss_i = pos_i * neg_i / (Sy_i * (L - Sy_i))
    out    = mean_i loss_i                      (scalar, float32)

Device strategy: shard the batch dim across 8 cores (2048 rows each). The
label masking is folded into the exponent: with s = M*y - c and M = 128,
    exp(-s)     = exp(c - M*y)     -> (1-y)*exp(c)   (y=1 underflows to 0)
    exp(s - M)  = exp(-c + M*(y-1))-> y*exp(-c)      (y=0 underflows to 0)
so ScalarE's fused activation-with-accumulate computes each masked row sum
in a single pass.

The host packs each [128, 1024] row-tile pair into one contiguous block:
per partition row, 4096 B of c (f32) followed by 1024 B of y (int8 - the
mask is 0/1 so the downcast is lossless and cuts DMA bytes by 37%). Each
tile arrives in a single 640 KB SWDGE DMA; the kernel bitcasts the two
regions back to f32 / int8 on-chip. Per tile the device does: one DVE
scalar_tensor_tensor (s = y*M - c), one DVE reduce_sum over y, and two
ScalarE exp+accum passes. Each core emits [3, 128, 16] row statistics
(pos, neg, Sy); the host finishes the tiny per-row division and the
global mean in float64.
"""

import numpy as np

B, L = 16384, 1024
N_CORES = 8
BS = B // N_CORES  # 2048 rows per core
P = 128
NSEG = BS // P  # 16 tiles of [128, L] per core
MASK = 128.0
ROWB = 4 * L + L  # bytes per partition row: c (f32) + y (int8)
DGE = "gpsimd"  # which engine issues the input loads: "gpsimd" or "sync"
IO_BUFS = 5


def _build_nc():
    import concourse.bacc as bacc
    import concourse.mybir as mybir
    from concourse.tile import TileContext

    f32 = mybir.dt.float32
    i8 = mybir.dt.int8
    u8 = mybir.dt.uint8

    nc = bacc.Bacc()
    cy_in = nc.dram_tensor("cy", [NSEG, P, ROWB], u8, kind="ExternalInput")
    stats = nc.dram_tensor("stats", [3, P, NSEG], f32, kind="ExternalOutput")

    with TileContext(nc) as tc:
        with (
            tc.tile_pool(name="io", bufs=IO_BUFS) as io,
            tc.tile_pool(name="psum_s", bufs=3, space="PSUM") as spool,
            tc.tile_pool(name="scratch", bufs=1, space="PSUM") as scratch,
            tc.tile_pool(name="accs", bufs=1) as accs,
        ):
            pos = accs.tile([P, NSEG], f32)
            neg = accs.tile([P, NSEG], f32)
            ysum = accs.tile([P, NSEG], f32)
            neg_mask = accs.tile([P, 1], f32)
            nc.vector.memset(neg_mask[:], -MASK)
            # Both exp passes dump their (unused) elementwise output here;
            # same-engine WAW ordering needs no semaphores.
            scr = scratch.tile([P, L], f32)

            dma_eng = nc.gpsimd if DGE == "gpsimd" else nc.sync
            for i in range(NSEG):
                t = io.tile([P, ROWB], u8, tag="cy")
                dma_eng.dma_start(t[:], cy_in[i])
                c_ap = t[:, 0 : 4 * L].bitcast(f32)
                y_ap = t[:, 4 * L : ROWB].bitcast(i8)

                s = spool.tile([P, L], f32, tag="s")
                nc.vector.scalar_tensor_tensor(
                    s[:],
                    y_ap,
                    MASK,
                    c_ap,
                    mybir.AluOpType.mult,
                    mybir.AluOpType.subtract,
                )
                nc.vector.reduce_sum(
                    ysum[:, i : i + 1], y_ap, axis=mybir.AxisListType.X
                )
                nc.scalar.activation(
                    scr[:],
                    s[:],
                    mybir.ActivationFunctionType.Exp,
                    scale=-1.0,
                    accum_out=neg[:, i : i + 1],
                )
                nc.scalar.activation(
                    scr[:],
                    s[:],
                    mybir.ActivationFunctionType.Exp,
                    bias=neg_mask[:],
                    scale=1.0,
                    accum_out=pos[:, i : i + 1],
                )

            nc.sync.dma_start(stats[0], pos[:])
            nc.sync.dma_start(stats[1], neg[:])
            nc.sync.dma_start(stats[2], ysum[:])

    nc.finalize()
    return nc


def _run(nc, in_maps, **kwargs):
    from concourse.bass_utils import run_bass_kernel_spmd

    return run_bass_kernel_spmd(nc, in_maps, list(range(N_CORES)), **kwargs)


def kernel(c, y, _bench_kwargs=None, _bench_result=None):
    c = np.asarray(c, dtype=np.float32)
    y = np.asarray(y, dtype=np.int32)
    assert c.shape == (B, L) and y.shape == (B, L)

    # Pack per [128, L] row-tile: per partition row 4096 B of c then 1024 B
    # of y as int8, so each tile is one contiguous 640 KB DMA.
    cyv = np.empty((N_CORES, NSEG, P, ROWB), np.uint8)
    cb = np.ascontiguousarray(c).view(np.uint8).reshape(N_CORES, NSEG, P, 4 * L)
    cyv[..., : 4 * L] = cb
    cyv[..., 4 * L :] = y.astype(np.uint8).reshape(N_CORES, NSEG, P, L)

    nc = _build_nc()
    in_maps = [{"cy": cyv[k]} for k in range(N_CORES)]
    res = _run(nc, in_maps, **(_bench_kwargs or {}))
    if _bench_result is not None:
        _bench_result.append(res)

    stats = np.stack([r["stats"] for r in res.results])  # [8, 3, 128, 16]
    pos = stats[:, 0].astype(np.float64)
    neg = stats[:, 1].astype(np.float64)
    sy = stats[:, 2].astype(np.float64)
    loss = pos * neg / (sy * (L - sy))
    return np.asarray(loss.mean(), dtype=np.float32)


# revision 23
# speedup vs baseline: 1.0513x; 1.0175x over previous
"""BPMLL loss kernel for Trainium2, data-parallel over 8 NeuronCores.

Reference computation (per sample row i of c [B, L], y [B, L] in {0,1}):
    pos_i  = sum_l y_il * exp(-c_il)
    neg_i  = sum_l (1 - y_il) * exp(c_il)
    Sy_i   = sum_l y_il
    loss_i = pos_i * neg_i / (Sy_i * (L - Sy_i))
    out    = mean_i loss_i                      (scalar, float32)

Device strategy: shard the batch dim across 8 cores (2048 rows each). The
label masking is folded into the exponent: with s = M*y - c and M = 128,
    exp(-s)     = exp(c - M*y)     -> (1-y)*exp(c)   (y=1 underflows to 0)
    exp(s - M)  = exp(-c + M*(y-1))-> y*exp(-c)      (y=0 underflows to 0)
so ScalarE's fused activation-with-accumulate computes each masked row sum
in a single pass.

The host packs each [128, 1024] row-tile pair into one contiguous block:
per partition row, 4096 B of c (f32) followed by 1024 B of y (int8 - the
mask is 0/1 so the downcast is lossless and cuts DMA bytes by 37%). Each
tile arrives in a single 640 KB SWDGE DMA; the kernel bitcasts the two
regions back to f32 / int8 on-chip. Per tile the device does: one DVE
scalar_tensor_tensor (s = y*M - c), one DVE reduce_sum over y, and two
ScalarE exp+accum passes. Each core emits [3, 128, 16] row statistics
(pos, neg, Sy); the host finishes the tiny per-row division and the
global mean in float64.
"""

import numpy as np

B, L = 16384, 1024
N_CORES = 8
BS = B // N_CORES  # 2048 rows per core
P = 128
NSEG = BS // P  # 16 tiles of [128, L] per core
MASK = 128.0
ROWB = 4 * L + L  # bytes per partition row: c (f32) + y (int8)
DGE = "gpsimd"  # which engine issues the input loads: "gpsimd" or "sync"
IO_BUFS = 3


def _build_nc():
    import concourse.bacc as bacc
    import concourse.mybir as mybir
    from concourse.tile import TileContext

    f32 = mybir.dt.float32
    i8 = mybir.dt.int8
    u8 = mybir.dt.uint8

    nc = bacc.Bacc()
    cy_in = nc.dram_tensor("cy", [NSEG, P, ROWB], u8, kind="ExternalInput")
    stats = nc.dram_tensor("stats", [3, P, NSEG], f32, kind="ExternalOutput")

    with TileContext(nc) as tc:
        with (
            tc.tile_pool(name="io", bufs=IO_BUFS) as io,
            tc.tile_pool(name="psum_s", bufs=3, space="PSUM") as spool,
            tc.tile_pool(name="scratch", bufs=1, space="PSUM") as scratch,
            tc.tile_pool(name="accs", bufs=1) as accs,
        ):
            pos = accs.tile([P, NSEG], f32)
            neg = accs.tile([P, NSEG], f32)
            ysum = accs.tile([P, NSEG], f32)
            neg_mask = accs.tile([P, 1], f32)
            nc.vector.memset(neg_mask[:], -MASK)
            # Both exp passes dump their (unused) elementwise output here;
            # same-engine WAW ordering needs no semaphores.
            scr = scratch.tile([P, L], f32)

            dma_eng = nc.gpsimd if DGE == "gpsimd" else nc.sync
            for i in range(NSEG):
                t = io.tile([P, ROWB], u8, tag="cy")
                dma_eng.dma_start(t[:], cy_in[i])
                c_ap = t[:, 0 : 4 * L].bitcast(f32)
                y_ap = t[:, 4 * L : ROWB].bitcast(i8)

                s = spool.tile([P, L], f32, tag="s")
                nc.vector.scalar_tensor_tensor(
                    s[:],
                    y_ap,
                    MASK,
                    c_ap,
                    mybir.AluOpType.mult,
                    mybir.AluOpType.subtract,
                )
                nc.vector.reduce_sum(
                    ysum[:, i : i + 1], y_ap, axis=mybir.AxisListType.X
                )
                nc.scalar.activation(
                    scr[:],
                    s[:],
                    mybir.ActivationFunctionType.Exp,
                    scale=-1.0,
                    accum_out=neg[:, i : i + 1],
                )
                nc.scalar.activation(
                    scr[:],
                    s[:],
                    mybir.ActivationFunctionType.Exp,
                    bias=neg_mask[:],
                    scale=1.0,
                    accum_out=pos[:, i : i + 1],
                )

            nc.sync.dma_start(stats[0], pos[:])
            nc.sync.dma_start(stats[1], neg[:])
            nc.sync.dma_start(stats[2], ysum[:])

    nc.finalize()
    return nc


def _run(nc, in_maps, **kwargs):
    from concourse.bass_utils import run_bass_kernel_spmd

    return run_bass_kernel_spmd(nc, in_maps, list(range(N_CORES)), **kwargs)


def kernel(c, y, _bench_kwargs=None, _bench_result=None):
    c = np.asarray(c, dtype=np.float32)
    y = np.asarray(y, dtype=np.int32)
    assert c.shape == (B, L) and y.shape == (B, L)

    # Pack per [128, L] row-tile: per partition row 4096 B of c then 1024 B
    # of y as int8, so each tile is one contiguous 640 KB DMA.
    cyv = np.empty((N_CORES, NSEG, P, ROWB), np.uint8)
    cb = np.ascontiguousarray(c).view(np.uint8).reshape(N_CORES, NSEG, P, 4 * L)
    cyv[..., : 4 * L] = cb
    cyv[..., 4 * L :] = y.astype(np.uint8).reshape(N_CORES, NSEG, P, L)

    nc = _build_nc()
    in_maps = [{"cy": cyv[k]} for k in range(N_CORES)]
    res = _run(nc, in_maps, **(_bench_kwargs or {}))
    if _bench_result is not None:
        _bench_result.append(res)

    stats = np.stack([r["stats"] for r in res.results])  # [8, 3, 128, 16]
    pos = stats[:, 0].astype(np.float64)
    neg = stats[:, 1].astype(np.float64)
    sy = stats[:, 2].astype(np.float64)
    loss = pos * neg / (sy * (L - sy))
    return np.asarray(loss.mean(), dtype=np.float32)
